# revision 1
# baseline (speedup 1.0000x reference)
"""Contrastive-loss kernel v2 for Trainium2 (8 NeuronCores, SPMD, raw Bass).

loss = sum_{i != j} dist[i,j] / (2 N (N-1)) collapses algebraically to
    total = (N-1)(Sx+Sy) - 2 sx.sy + 2 tr
with Sx = sum x^2, sx = column sums, tr = sum_i x_i.y_i. Each core reads
its 1/8 row-shard of both tensors and returns tiny partials.

v2 layout (probe-driven):
  - x (512 KiB) loads on the SP HWDGE ring (the only ring whose data can
    flow before the ACT ring unblocks); y loads via gpsimd SWDGE, which
    generates descriptors independently, CAST to bf16 in the DMA datapath
    (halves PE/ACT work for y at no DMA cost).
  - ACT: Square+accum row-sums of x^2 / y^2. DVE: fused x*y multiply+
    row-sum (tensor_tensor_reduce). PE: ones^T matmuls accumulate column
    sums of x / y into single PSUM banks (both halves of the free dim
    fold into [1,512]); one tiny matmul collapses the [128,3] row-sum
    tile to [1,3].
  - PSUM->SBUF copies run on ACT and DVE in parallel; one [1,1027] out
    DMA on the idle SP ring. Block(no_gpsimd_drain=True) skips the
    expensive gpsimd dge_drain in the epilogue.
"""

import numpy as np

N, D = 8192, 128
NCORES = 8
ROWS = N // NCORES          # 1024 rows per core per tensor
P = 128
KG = ROWS // P              # 8 row-groups folded into the free dim
FREE = KG * D               # 1024 free elements per partition
HALF = FREE // 2            # 512 = one PSUM bank of f32
OUT_LEN = 2 * HALF + 3      # [cols_x(512) | cols_y(512) | Sx, Sy, tr]

CAST_Y = True               # y loads as bf16 via SWDGE cast

_NC_CACHE = {}


def _build_bass():
    from contextlib import ExitStack

    import concourse.bass as bass
    from concourse import mybir

    f32 = mybir.dt.float32
    bf16 = mybir.dt.bfloat16
    ydt = bf16 if CAST_Y else f32
    SQ = mybir.ActivationFunctionType.Square
    MUL = mybir.AluOpType.mult
    ADD = mybir.AluOpType.add
    nc = bass.Bass()
    x = nc.dram_tensor("x", [ROWS, D], f32, kind="ExternalInput")
    y = nc.dram_tensor("y", [ROWS, D], f32, kind="ExternalInput")
    out = nc.dram_tensor("out", [1, OUT_LEN], f32, kind="ExternalOutput")

    xr = x.rearrange("(p k) d -> p (k d)", p=P)
    yr = y.rearrange("(p k) d -> p (k d)", p=P)

    ones_f = nc.const_aps.tensor(1.0, (P, 1), f32)
    ones_y = nc.const_aps.tensor(1.0, (P, 1), ydt)

    with ExitStack() as ctx:
        X = ctx.enter_context(nc.sbuf_tensor("X", [P, FREE], f32))
        Y = ctx.enter_context(nc.sbuf_tensor("Y", [P, FREE], ydt))
        scr = ctx.enter_context(nc.sbuf_tensor("scr", [P, FREE], f32))
        warm = ctx.enter_context(nc.sbuf_tensor("warm", [P, 1], f32))
        rs = ctx.enter_context(nc.sbuf_tensor("rs", [P, 3], f32))
        outsb = ctx.enter_context(nc.sbuf_tensor("outsb", [1, OUT_LEN], f32))
        px = ctx.enter_context(nc.psum_tensor([1, HALF], f32))
        py = ctx.enter_context(nc.psum_tensor([1, HALF], f32))
        prs = ctx.enter_context(nc.psum_tensor([1, 3], f32))
        pwarm = ctx.enter_context(nc.psum_tensor([1, 1], f32))

        dx = ctx.enter_context(nc.semaphore("dx"))
        dy = ctx.enter_context(nc.semaphore("dy"))
        dout = ctx.enter_context(nc.semaphore("dout"))
        pe_sem = ctx.enter_context(nc.semaphore("pe_sem"))
        a_sem = ctx.enter_context(nc.semaphore("a_sem"))
        v_sem = ctx.enter_context(nc.semaphore("v_sem"))
        copy_sem = ctx.enter_context(nc.semaphore("copy_sem"))

        with nc.Block() as block:

            @block.sync
            def _(sync):
                sync.dma_start(out=X[:], in_=xr).then_inc(dx, 16)
                sync.wait_ge(copy_sem, 3)
                sync.dma_start(out=out[:, :], in_=outsb[:]).then_inc(dout, 16)
                sync.wait_ge(dout, 16)

            @block.gpsimd
            def _(gpsimd):
                gpsimd.dma_start(out=Y[:], in_=yr).then_inc(dy, 16)

            @block.tensor
            def _(tensor):
                # warmup matmul: opens the PE HAM clock gate early
                nc.tensor.matmul(pwarm[:], ones_f, ones_f[:, 0:1],
                                 start=True, stop=True).then_inc(pe_sem, 1)
                # keep the PE HAM clock gate open until data lands
                nc.tensor.matmul(pwarm[:], ones_f, warm[:],
                                 start=True, stop=True).then_inc(pe_sem, 1)
                nc.tensor.matmul(pwarm[:], ones_f, warm[:],
                                 start=True, stop=True).then_inc(pe_sem, 1)
                tensor.wait_ge(dy, 16)
                nc.tensor.matmul(py[:], ones_y, Y[:, 0:HALF],
                                 start=True, stop=False).then_inc(pe_sem, 1)
                nc.tensor.matmul(py[:], ones_y, Y[:, HALF:FREE],
                                 start=False, stop=True).then_inc(pe_sem, 1)
                tensor.wait_ge(dx, 16)
                nc.tensor.matmul(px[:], ones_f, X[:, 0:HALF],
                                 start=True, stop=False).then_inc(pe_sem, 1)
                nc.tensor.matmul(px[:], ones_f, X[:, HALF:FREE],
                                 start=False, stop=True).then_inc(pe_sem, 1)
                tensor.wait_ge(a_sem, 2)
                tensor.wait_ge(v_sem, 1)
                nc.tensor.matmul(prs[:], ones_f, rs[:],
                                 start=True, stop=True).then_inc(pe_sem, 1)

            @block.scalar
            def _(scalar):
                # Prewarm the Square PWP table while the DMAs fly.
                nc.scalar.activation(out=warm[:], in_=warm[:], func=SQ)
                scalar.wait_ge(dx, 16)
                nc.scalar.activation(out=scr[:], in_=X[:], func=SQ,
                                     accum_out=rs[:, 0:1]).then_inc(a_sem, 1)
                scalar.wait_ge(dy, 16)
                nc.scalar.activation(out=scr[:], in_=Y[:], func=SQ,
                                     accum_out=rs[:, 1:2]).then_inc(a_sem, 1)
                scalar.wait_ge(pe_sem, 7)
                nc.scalar.copy(out=outsb[0:1, 0:HALF],
                               in_=px[:]).then_inc(copy_sem, 1)
                scalar.wait_ge(pe_sem, 8)
                nc.scalar.copy(out=outsb[0:1, 2 * HALF:OUT_LEN],
                               in_=prs[:]).then_inc(copy_sem, 1)

            @block.vector
            def _(vector):
                vector.wait_ge(dx, 16)
                vector.wait_ge(dy, 16)
                nc.vector.tensor_mul(out=scr[:], in0=X[:], in1=Y[:])
                nc.vector.reduce_sum(rs[:, 2:3], scr[:],
                                     axis=mybir.AxisListType.X).then_inc(
                    v_sem, 1)
                vector.wait_ge(pe_sem, 5)
                nc.vector.tensor_copy(out=outsb[0:1, HALF:2 * HALF],
                                      in_=py[:]).then_inc(copy_sem, 1)

    return nc


def _get_nc():
    if "nc" not in _NC_CACHE:
        _NC_CACHE["nc"] = _build_bass()
    return _NC_CACHE["nc"]


def _run_device(f1, f2, **spmd_kwargs):
    from concourse.bass_utils import run_bass_kernel_spmd

    nc = _get_nc()
    in_maps = [
        {"x": f1[c * ROWS:(c + 1) * ROWS], "y": f2[c * ROWS:(c + 1) * ROWS]}
        for c in range(NCORES)
    ]
    return run_bass_kernel_spmd(nc, in_maps, core_ids=list(range(NCORES)),
                                **spmd_kwargs)


def _combine(results):
    sx = np.zeros(D, np.float64)
    sy = np.zeros(D, np.float64)
    Sx = Sy = tr = 0.0
    for r in results:
        o = r["out"][0].astype(np.float64)
        # px[0,(k',d)] folds row-groups k' and k'+4 -> sum the 4 groups
        sx += o[0:HALF].reshape(HALF // D, D).sum(axis=0)
        sy += o[HALF:2 * HALF].reshape(HALF // D, D).sum(axis=0)
        Sx += o[2 * HALF]
        Sy += o[2 * HALF + 1]
        tr += o[2 * HALF + 2]
    total = (N - 1) * (Sx + Sy) - 2.0 * float(sx @ sy) + 2.0 * tr
    loss = total / 2.0 / (N * (N - 1))
    return np.asarray(loss, dtype=np.float32)


def kernel(feature1, feature2, label=None, **_unused):
    f1 = np.ascontiguousarray(np.asarray(feature1, dtype=np.float32))
    f2 = np.ascontiguousarray(np.asarray(feature2, dtype=np.float32))
    res = _run_device(f1, f2)
    return _combine(res.results)



# revision 15
# speedup vs baseline: 1.1580x; 1.1580x over previous
"""Contrastive-loss kernel v3 for Trainium2 (8 NeuronCores, SPMD, raw Bass).

loss = sum_{i != j} dist[i,j] / (2 N (N-1)) collapses algebraically to
    total = (N-1)(Sx+Sy) - 2 sx.sy + 2 tr
with Sx = sum x^2, sx = column sums, tr = sum_i x_i.y_i. Each core reads
its 1/8 row-shard of both tensors and returns tiny partials.

v3 layout (trace-driven redesign of v2):
  - Both tensors load f32 over the TWO HWDGE rings (x on qSP, y on qACT),
    each split into 4 free-dim chunks so compute starts while the bus is
    still draining. No gpsimd/SWDGE anywhere (saves the slow DSP branch +
    descriptor-gen latency of v2).
  - PE: column sums via ones^T @ chunk in float32r (single-pass fp32:
    1 cycle/row for moving free >= 256, vs 4 for plain fp32's LOW/HIGH
    split). Chunks accumulate into one [1,256] PSUM bank per tensor.
  - DVE: fused multiply+row-reduce (tensor_tensor_reduce) per chunk for
    x*y and y*y partials; then copies both PSUM colsum rows to SBUF.
  - ACT: two Square+accum passes over x halves (PWP table load hides
    behind the DMA wait), then idles.
  - Out: two small DMAs on the idle SP ring (rs [128,10] + colsums
    [1,512]); per-partition partials finish on the host.
"""

import numpy as np

N, D = 8192, 128
NCORES = 8
ROWS = N // NCORES          # 1024 rows per core per tensor
P = 128
KG = ROWS // P              # 8 row-groups folded into the free dim
FREE = KG * D               # 1024 free elements per partition
NCH = 4                     # chunks per tensor
CH = FREE // NCH            # 256 free elements per chunk (2 row-groups)
CSUM = 2 * CH               # colsum sbuf row: [px(256) | py(256)]

WAIT_DOUT = True            # wait for the output DMA semaphore in-kernel

_NC_CACHE = {}


def _build_bass():
    from contextlib import ExitStack

    import concourse.bass as bass
    from concourse import mybir

    f32 = mybir.dt.float32
    f32r = mybir.dt.float32r
    SQ = mybir.ActivationFunctionType.Square
    MUL = mybir.AluOpType.mult
    ADD = mybir.AluOpType.add
    nc = bass.Bass()
    x = nc.dram_tensor("x", [ROWS, D], f32, kind="ExternalInput")
    y = nc.dram_tensor("y", [ROWS, D], f32, kind="ExternalInput")
    rs_out = nc.dram_tensor("rs_out", [P, 10], f32, kind="ExternalOutput")
    cols_out = nc.dram_tensor("cols_out", [1, CSUM], f32, kind="ExternalOutput")

    xr = x.rearrange("(p k) d -> p (k d)", p=P)
    yr = y.rearrange("(p k) d -> p (k d)", p=P)

    ones_f = nc.const_aps.tensor(1.0, (P, 1), f32)
    ones_r = ones_f.bitcast(f32r)

    with ExitStack() as ctx:
        X = ctx.enter_context(nc.sbuf_tensor("X", [P, FREE], f32))
        Y = ctx.enter_context(nc.sbuf_tensor("Y", [P, FREE], f32))
        scrA = ctx.enter_context(nc.sbuf_tensor("scrA", [P, 2 * CH], f32))
        scrV = ctx.enter_context(nc.sbuf_tensor("scrV", [P, CH], f32))
        warm = ctx.enter_context(nc.sbuf_tensor("warm", [P, 1], f32))
        rs = ctx.enter_context(nc.sbuf_tensor("rs", [P, 10], f32))
        colsb = ctx.enter_context(nc.sbuf_tensor("colsb", [1, CSUM], f32))
        px = ctx.enter_context(nc.psum_tensor([1, CH], f32))
        py = ctx.enter_context(nc.psum_tensor([1, CH], f32))
        pwarm = ctx.enter_context(nc.psum_tensor([1, 1], f32))

        dx = ctx.enter_context(nc.semaphore("dx"))
        dy = ctx.enter_context(nc.semaphore("dy"))
        dout = ctx.enter_context(nc.semaphore("dout"))
        pe_sem = ctx.enter_context(nc.semaphore("pe_sem"))
        a_sem = ctx.enter_context(nc.semaphore("a_sem"))
        v_sem = ctx.enter_context(nc.semaphore("v_sem"))

        with nc.Block() as block:

            @block.sync
            def _(sync):
                for c in range(NCH):
                    # f32r-tagged copy (same bytes) so the BIR verifier
                    # accepts the fp32r matmul consumers downstream.
                    sync.dma_start(
                        out=X[:, c * CH:(c + 1) * CH].bitcast(f32r),
                        in_=xr[:, c * CH:(c + 1) * CH].bitcast(f32r),
                    ).then_inc(dx, 16)
                # rs tile: ACT x^2 cols (a_sem 2) + DVE ttr cols (v_sem 1)
                sync.wait_ge(a_sem, 2)
                sync.wait_ge(v_sem, 1)
                sync.dma_start(out=rs_out[:, :], in_=rs[:]).then_inc(dout, 16)
                # colsum row: ACT px copy (a_sem 3) + DVE py copy (v_sem 2)
                sync.wait_ge(a_sem, 3)
                sync.wait_ge(v_sem, 2)
                sync.dma_start(out=cols_out[:, :], in_=colsb[:]).then_inc(
                    dout, 16)
                if WAIT_DOUT:
                    sync.wait_ge(dout, 32)

            @block.scalar
            def _(scalar):
                for c in range(NCH):
                    scalar.dma_start(
                        out=Y[:, c * CH:(c + 1) * CH].bitcast(f32r),
                        in_=yr[:, c * CH:(c + 1) * CH].bitcast(f32r),
                    ).then_inc(dy, 16)
                # Prewarm the Square PWP table while the DMAs fly.
                nc.scalar.activation(out=warm[:], in_=warm[:], func=SQ)
                scalar.wait_ge(dx, 32)
                nc.scalar.activation(out=scrA[:], in_=X[:, 0:2 * CH], func=SQ,
                                     accum_out=rs[:, 0:1]).then_inc(a_sem, 1)
                scalar.wait_ge(dx, 64)
                nc.scalar.activation(out=scrA[:], in_=X[:, 2 * CH:FREE],
                                     func=SQ,
                                     accum_out=rs[:, 1:2]).then_inc(a_sem, 1)
                scalar.wait_ge(pe_sem, 1)
                nc.scalar.copy(out=colsb[0:1, 0:CH],
                               in_=px[:]).then_inc(a_sem, 1)

            @block.vector
            def _(vector):
                for c in range(NCH):
                    vector.wait_ge(dx, 16 * (c + 1))
                    vector.wait_ge(dy, 16 * (c + 1))
                    # fused multiply + row-sum: out=(in0*1)*in1, accum=sum
                    nc.vector.scalar_tensor_tensor(
                        out=scrV[:], in0=X[:, c * CH:(c + 1) * CH], scalar=1.0,
                        in1=Y[:, c * CH:(c + 1) * CH], op0=MUL, op1=MUL,
                        accum_out=rs[:, 2 + c:3 + c])
                    inst = nc.vector.scalar_tensor_tensor(
                        out=scrV[:], in0=Y[:, c * CH:(c + 1) * CH], scalar=1.0,
                        in1=Y[:, c * CH:(c + 1) * CH], op0=MUL, op1=MUL,
                        accum_out=rs[:, 6 + c:7 + c])
                    if c == NCH - 1:
                        inst.then_inc(v_sem, 1)
                vector.wait_ge(pe_sem, 2)
                nc.vector.tensor_copy(out=colsb[0:1, CH:CSUM],
                                      in_=py[:]).then_inc(v_sem, 1)

            @block.tensor
            def _(tensor):
                # warmup matmuls: open the PE HAM clock gate early
                nc.tensor.matmul(pwarm[:], ones_f, ones_f[:, 0:1],
                                 start=True, stop=True)
                nc.tensor.matmul(pwarm[:], ones_f, warm[:],
                                 start=True, stop=True)
                nc.tensor.matmul(pwarm[:], ones_f, warm[:],
                                 start=True, stop=True)
                for c in range(NCH):
                    tensor.wait_ge(dx, 16 * (c + 1))
                    inst = nc.tensor.matmul(
                        px[:], ones_r,
                        X[:, c * CH:(c + 1) * CH].bitcast(f32r),
                        start=(c == 0), stop=(c == NCH - 1))
                    if c == NCH - 1:
                        inst.then_inc(pe_sem, 1)
                    tensor.wait_ge(dy, 16 * (c + 1))
                    inst = nc.tensor.matmul(
                        py[:], ones_r,
                        Y[:, c * CH:(c + 1) * CH].bitcast(f32r),
                        start=(c == 0), stop=(c == NCH - 1))
                    if c == NCH - 1:
                        inst.then_inc(pe_sem, 1)

    return nc


def _get_nc():
    if "nc" not in _NC_CACHE:
        _NC_CACHE["nc"] = _build_bass()
    return _NC_CACHE["nc"]


def _run_device(f1, f2, **spmd_kwargs):
    from concourse.bass_utils import run_bass_kernel_spmd

    nc = _get_nc()
    in_maps = [
        {"x": f1[c * ROWS:(c + 1) * ROWS], "y": f2[c * ROWS:(c + 1) * ROWS]}
        for c in range(NCORES)
    ]
    return run_bass_kernel_spmd(nc, in_maps, core_ids=list(range(NCORES)),
                                **spmd_kwargs)


def _combine(results):
    sx = np.zeros(D, np.float64)
    sy = np.zeros(D, np.float64)
    Sx = Sy = tr = 0.0
    for r in results:
        rsm = r["rs_out"].astype(np.float64)      # [128, 10]
        cb = r["cols_out"][0].astype(np.float64)  # [512] = px(256)|py(256)
        Sx += rsm[:, 0:2].sum()
        tr += rsm[:, 2:6].sum()
        Sy += rsm[:, 6:10].sum()
        # px[j] folds even k-groups (j<128) and odd (j>=128); same for py.
        sx += cb[0:D] + cb[D:CH]
        sy += cb[CH:CH + D] + cb[CH + D:CSUM]
    total = (N - 1) * (Sx + Sy) - 2.0 * float(sx @ sy) + 2.0 * tr
    loss = total / 2.0 / (N * (N - 1))
    return np.asarray(loss, dtype=np.float32)


def kernel(feature1, feature2, label=None, **_unused):
    f1 = np.ascontiguousarray(np.asarray(feature1, dtype=np.float32))
    f2 = np.ascontiguousarray(np.asarray(feature2, dtype=np.float32))
    res = _run_device(f1, f2)
    return _combine(res.results)
